# revision 2
# baseline (speedup 1.0000x reference)
"""Cross-temporal attention Trainium2 (Bass/Tile) kernel.

Problem: two streams x1, x2 of shape [B=4, C=256, H=64, W=64]; tokens are the
H*W=4096 spatial positions. Per batch b and stream s:
    q_s = t_s @ Wq.T + bq ; k_s = t_s @ Wk.T + bk ; v_s = t_s @ Wv.T + bv
    out_s = softmax(q_s @ k_{3-s}.T) @ v_s            (no 1/sqrt(d) scaling)

Sharding: 8 NeuronCores, one (batch, stream) unit per core (4 batches x 2
streams). Fully SPMD — the same program runs on every core, only the input
bindings differ. No collectives.

Per-core layout trick: x[b] is already [C, N] channel-major, which is exactly
the transposed token matrix. All intermediates stay transposed:
    QT = Wq @ X + bq   [C, N]      (PE: lhsT = Wq^T chunks, rhs = X chunks)
    KT = Wk @ Xo + bk  [C, N]
    V  = X^T @ Wv^T + bv  [N, C]   (PE: lhsT = X chunks, rhs = Wv^T)
    ST = KT^T-block @ QT = scores^T  [m, n] blocks   (softmax over m = partitions)
    E  = exp(ST)   (no max subtraction: |logits| < ~40 << 88, fp32-safe)
    U  = accum_m V^T-block @ E  -> [C, n] unnormalized out^T
    D  = column sums of E (ones-matmul replicates to all partitions)
    OT = U / D     [C, N] == x_out[b] flattened. No transposes anywhere.

All matmuls run in float32r (TF32) at 1 cycle/row; data is DMA'd straight into
float32r tiles (PE rounds internally) or produced by compute ops with float32r
output dtype.
"""

import numpy as np

import concourse.bacc as bacc
import concourse.mybir as mybir
import concourse.tile as tile
from concourse.bass_utils import run_bass_kernel_spmd

F32 = mybir.dt.float32
F32R = mybir.dt.float32r
AF = mybir.ActivationFunctionType

B, C, H, W = 4, 256, 64, 64
N = H * W            # 4096 tokens
CK = C // 128        # 2 channel chunks of 128
NT = 512             # attention n-tile (query block, free dim)
N_NT = N // NT       # 8
MB = 128             # key/value block (partition block)
N_MB = N // MB       # 32
SKEW = 2             # software-pipeline skew between S and U matmuls

_NC_CACHE = None
LAST_RESULT = None   # BassKernelResults of the most recent kernel() call


def _build():
    nc = bacc.Bacc("TRN2", target_bir_lowering=False, debug=False)

    xa = nc.dram_tensor("xa", [C, N], F32, kind="ExternalInput").ap()
    xb = nc.dram_tensor("xb", [C, N], F32, kind="ExternalInput").ap()
    wq = nc.dram_tensor("wq_t", [C, C], F32, kind="ExternalInput").ap()
    wk = nc.dram_tensor("wk_t", [C, C], F32, kind="ExternalInput").ap()
    wv = nc.dram_tensor("wv_t", [C, C], F32, kind="ExternalInput").ap()
    bq = nc.dram_tensor("bq", [C, 1], F32, kind="ExternalInput").ap()
    bk = nc.dram_tensor("bk", [C, 1], F32, kind="ExternalInput").ap()
    bv = nc.dram_tensor("bv", [1, C], F32, kind="ExternalInput").ap()
    out = nc.dram_tensor("o", [C, N], F32, kind="ExternalOutput").ap()

    with tile.TileContext(nc) as tc:
        with tc.tile_pool(name="persist", bufs=1) as pp, \
             tc.tile_pool(name="work", bufs=2) as wp:
            # ---- constants & parameters -------------------------------
            ones_f = pp.tile([128, 128], F32, tag="ones_f")
            nc.vector.memset(ones_f[:], 1.0)
            ones_r = pp.tile([128, 128], F32R, tag="ones_r")
            nc.vector.tensor_copy(ones_r[:], ones_f[:])

            w_r = {}
            for name, src in (("wq", wq), ("wk", wk), ("wv", wv)):
                t = pp.tile([128, CK, C], F32R, tag=f"{name}_r")
                nc.sync.dma_start(
                    t[:], src.rearrange("(k p) m -> p k m", p=128).bitcast(F32R))
                w_r[name] = t
            bq_sb = pp.tile([128, CK, 1], F32, tag="bq_sb")
            nc.sync.dma_start(bq_sb[:], bq.rearrange("(c p) o -> p c o", p=128))
            bk_sb = pp.tile([128, CK, 1], F32, tag="bk_sb")
            nc.sync.dma_start(bk_sb[:], bk.rearrange("(c p) o -> p c o", p=128))
            bv_r = pp.tile([1, C], F32R, tag="bv_r")
            nc.sync.dma_start(bv_r[:], bv.bitcast(F32R))

            # ---- stream inputs (channel-chunked, straight to f32r) ----
            xa_r = pp.tile([128, CK, N], F32R, tag="xa_r")
            xb_r = pp.tile([128, CK, N], F32R, tag="xb_r")
            for ki in range(CK):
                nc.sync.dma_start(xa_r[:, ki, :],
                                  xa[ki * 128:(ki + 1) * 128, :].bitcast(F32R))
            for ki in range(CK):
                nc.sync.dma_start(xb_r[:, ki, :],
                                  xb[ki * 128:(ki + 1) * 128, :].bitcast(F32R))

            qt = pp.tile([128, CK, N], F32R, tag="qt")    # QT[c, n] co-chunked
            kt = pp.tile([128, CK, N], F32R, tag="kt")    # KT[c, m]
            v_r = pp.tile([128, CK, N], F32R, tag="v_r")  # V[m, c] -> [p, co, mb*128+j]

            # PSUM pools shared by both phases: s(3 banks) + u(4) + d(1) = 8
            with tc.tile_pool(name="s_ps", bufs=3, space="PSUM") as sp, \
                 tc.tile_pool(name="u_ps", bufs=2, space="PSUM") as up, \
                 tc.tile_pool(name="d_ps", bufs=1, space="PSUM") as dp, \
                 tc.tile_pool(name="e_sb", bufs=4) as ep, \
                 tc.tile_pool(name="acc", bufs=2) as ap_:
                # ---- phase 1: projections -----------------------------
                # QT then KT: for each output-channel chunk and n tile
                for dst, w_t, b_sb, src in ((qt, w_r["wq"], bq_sb, xa_r),
                                            (kt, w_r["wk"], bk_sb, xb_r)):
                    for co in range(CK):
                        for nt in range(N_NT):
                            ps = sp.tile([128, NT], F32, tag="s")
                            for ki in range(CK):
                                nc.tensor.matmul(
                                    ps[:],
                                    w_t[:, ki, co * 128:(co + 1) * 128],
                                    src[:, ki, nt * NT:(nt + 1) * NT],
                                    start=(ki == 0), stop=(ki == CK - 1))
                            nc.vector.tensor_scalar_add(
                                dst[:, co, nt * NT:(nt + 1) * NT], ps[:],
                                b_sb[:, co, :])
                # V: token-major blocks, bias folded in via K=1 ones matmul
                for mb in range(N_MB):
                    ps = sp.tile([128, NT], F32, tag="s")
                    for ki in range(CK):
                        nc.tensor.matmul(
                            ps[:, 0:C],
                            xa_r[:, ki, mb * 128:(mb + 1) * 128],
                            w_r["wv"][:, ki, :],
                            start=(ki == 0), stop=False)
                    nc.tensor.matmul(ps[:, 0:C], ones_r[0:1, :], bv_r[:],
                                     start=False, stop=True)
                    nc.vector.tensor_copy(
                        v_r[:, :, mb * 128:(mb + 1) * 128],
                        ps[:, 0:C].rearrange("p (c j) -> p c j", c=CK))

                # ---- phase 2: attention -------------------------------
                for nt in range(N_NT):
                    n_sl = slice(nt * NT, (nt + 1) * NT)
                    u_ps = up.tile([128, CK, NT], F32, tag="u")
                    dacc = ap_.tile([128, NT], F32, tag="dacc")
                    e_tiles = {}
                    s_tiles = {}
                    for step in range(N_MB + SKEW):
                        # S + exp side, runs SKEW blocks ahead
                        if step < N_MB:
                            mb = step
                            s_ps = sp.tile([128, NT], F32, tag="s")
                            for co in range(CK):
                                nc.tensor.matmul(
                                    s_ps[:],
                                    kt[:, co, mb * 128:(mb + 1) * 128],
                                    qt[:, co, n_sl],
                                    start=(co == 0), stop=(co == CK - 1))
                            e_r = ep.tile([128, NT], F32R, tag="e")
                            nc.scalar.activation(e_r[:], s_ps[:], AF.Exp)
                            s_tiles[mb] = s_ps
                            e_tiles[mb] = e_r
                        # U accumulation + D accumulation side
                        if step >= SKEW:
                            mb = step - SKEW
                            e_r = e_tiles.pop(mb)
                            for co in range(CK):
                                nc.tensor.matmul(
                                    u_ps[:, co, :],
                                    v_r[:, co, mb * 128:(mb + 1) * 128],
                                    e_r[:],
                                    start=(mb == 0), stop=(mb == N_MB - 1))
                            if mb == 0:
                                nc.vector.tensor_copy(dacc[:], e_r[:])
                            else:
                                nc.vector.tensor_add(dacc[:], dacc[:], e_r[:])
                    # normalize: D replicated to all partitions via ones matmul
                    dacc_r = ap_.tile([128, NT], F32R, tag="dacc_r")
                    nc.vector.tensor_copy(dacc_r[:], dacc[:])
                    d_ps = dp.tile([128, NT], F32, tag="d")
                    nc.tensor.matmul(d_ps[:], ones_r[:], dacc_r[:],
                                     start=True, stop=True)
                    dinv = ap_.tile([128, NT], F32, tag="dinv")
                    nc.vector.reciprocal(dinv[:], d_ps[:])
                    for co in range(CK):
                        o_sb = wp.tile([128, NT], F32, tag="o_sb")
                        nc.vector.tensor_mul(o_sb[:], u_ps[:, co, :], dinv[:])
                        nc.sync.dma_start(
                            out[co * 128:(co + 1) * 128, n_sl], o_sb[:])
    nc.compile()
    return nc


def _get_nc():
    global _NC_CACHE
    if _NC_CACHE is None:
        _NC_CACHE = _build()
    return _NC_CACHE


def kernel(x1, x2, Wq, bq, Wk, bk, Wv, bv):
    global LAST_RESULT
    x1 = np.asarray(x1, dtype=np.float32)
    x2 = np.asarray(x2, dtype=np.float32)
    shared = {
        "wq_t": np.ascontiguousarray(np.asarray(Wq, np.float32).T),
        "wk_t": np.ascontiguousarray(np.asarray(Wk, np.float32).T),
        "wv_t": np.ascontiguousarray(np.asarray(Wv, np.float32).T),
        "bq": np.asarray(bq, np.float32).reshape(C, 1),
        "bk": np.asarray(bk, np.float32).reshape(C, 1),
        "bv": np.asarray(bv, np.float32).reshape(1, C),
    }
    in_maps = []
    for core in range(8):
        b, s = core % B, core // B
        xs, xo = (x1, x2) if s == 0 else (x2, x1)
        in_maps.append({
            "xa": np.ascontiguousarray(xs[b].reshape(C, N)),
            "xb": np.ascontiguousarray(xo[b].reshape(C, N)),
            **shared,
        })
    nc = _get_nc()
    res = run_bass_kernel_spmd(nc, in_maps, list(range(8)))
    LAST_RESULT = res
    x1_out = np.stack([res.results[b]["o"].reshape(C, H, W) for b in range(B)])
    x2_out = np.stack([res.results[B + b]["o"].reshape(C, H, W) for b in range(B)])
    return (x1_out, x2_out)


# revision 5
# speedup vs baseline: 1.0243x; 1.0243x over previous
"""Cross-temporal attention Trainium2 (Bass/Tile) kernel.

Problem: two streams x1, x2 of shape [B=4, C=256, H=64, W=64]; tokens are the
H*W=4096 spatial positions. Per batch b and stream s:
    q_s = t_s @ Wq.T + bq ; k_s = t_s @ Wk.T + bk ; v_s = t_s @ Wv.T + bv
    out_s = softmax(q_s @ k_{3-s}.T) @ v_s            (no 1/sqrt(d) scaling)

Sharding: 8 NeuronCores, one (batch, stream) unit per core (4 batches x 2
streams). Fully SPMD — the same program runs on every core, only the input
bindings differ. No collectives.

Per-core layout trick: x[b] is already [C, N] channel-major, which is exactly
the transposed token matrix. All intermediates stay transposed:
    QT = Wq @ X + bq   [C, N]      (PE: lhsT = Wq^T chunks, rhs = X chunks)
    KT = Wk @ Xo + bk  [C, N]
    V  = X^T @ Wv^T + bv  [N, C]   (PE: lhsT = X chunks, rhs = Wv^T)
    ST = KT^T-block @ QT = scores^T  [m, n] blocks   (softmax over m = partitions)
    E  = exp(ST)   (no max subtraction: |logits| < ~40 << 88, fp32-safe)
    U  = accum_m V^T-block @ E  -> [C, n] unnormalized out^T
    D  = column sums of E (ones-matmul replicates to all partitions)
    OT = U / D     [C, N] == x_out[b] flattened. No transposes anywhere.

All matmuls run in float32r (TF32) at 1 cycle/row; data is DMA'd straight into
float32r tiles (PE rounds internally) or produced by compute ops with float32r
output dtype.

v2 notes (driven by NTFF trace of v1, 355us):
 - attention processes n-tile PAIRS (1024 columns): one exp + one dacc add per
   key block covers both tiles (fewer, wider ACT/DVE ops).
 - V-projection matmuls (256-free, LDWEIGHTS-bound) are interleaved between
   QT/KT groups so the PE never micro-idles long enough to re-arm the HAM
   throttle (v1 lost ~37us to a K=4/8 window at the phase boundary).
 - input DMAs split into 512KB pieces for queue parallelism (v1 stalled ~20us
   on the first 2MB DMA); xb is streamed through a small pool, not resident.
 - reciprocal -> reciprocal_approx_fast (denominators need ~18 bits).
"""

import numpy as np

import concourse.bacc as bacc
import concourse.mybir as mybir
import concourse.tile as tile
from concourse.bass_utils import run_bass_kernel_spmd

F32 = mybir.dt.float32
F32R = mybir.dt.float32r
AF = mybir.ActivationFunctionType

B, C, H, W = 4, 256, 64, 64
N = H * W            # 4096 tokens
CK = C // 128        # 2 channel chunks of 128
NT = 512             # attention n-tile (query block, free dim)
NP = 1024            # n-tile pair width
N_PAIR = N // NP     # 4
MB = 128             # key/value block (partition block)
N_MB = N // MB       # 32
SKEW = 2             # software-pipeline skew between S and U matmuls

_NC_CACHE = None
LAST_RESULT = None   # BassKernelResults of the most recent kernel() call


def _build():
    nc = bacc.Bacc("TRN2", target_bir_lowering=False, debug=False)

    xa = nc.dram_tensor("xa", [C, N], F32, kind="ExternalInput").ap()
    xb = nc.dram_tensor("xb", [C, N], F32, kind="ExternalInput").ap()
    wq = nc.dram_tensor("wq_t", [C, C], F32, kind="ExternalInput").ap()
    wk = nc.dram_tensor("wk_t", [C, C], F32, kind="ExternalInput").ap()
    wv = nc.dram_tensor("wv_t", [C, C], F32, kind="ExternalInput").ap()
    bq = nc.dram_tensor("bq", [C, 1], F32, kind="ExternalInput").ap()
    bk = nc.dram_tensor("bk", [C, 1], F32, kind="ExternalInput").ap()
    bv = nc.dram_tensor("bv", [1, C], F32, kind="ExternalInput").ap()
    out = nc.dram_tensor("o", [C, N], F32, kind="ExternalOutput").ap()

    with tile.TileContext(nc) as tc:
        with tc.tile_pool(name="persist", bufs=1) as pp, \
             tc.tile_pool(name="xbs", bufs=4) as xbp, \
             tc.tile_pool(name="os", bufs=2) as op_, \
             tc.tile_pool(name="s_ps", bufs=2, space="PSUM") as sp, \
             tc.tile_pool(name="u_ps", bufs=1, space="PSUM") as up, \
             tc.tile_pool(name="e_sb", bufs=3) as ep, \
             tc.tile_pool(name="acc", bufs=1) as ap_:
            # ---- constants & parameters -------------------------------
            ones_f = pp.tile([128, 128], F32, tag="ones_f")
            nc.vector.memset(ones_f[:], 1.0)
            ones_r = pp.tile([128, 128], F32R, tag="ones_r")
            nc.vector.tensor_copy(ones_r[:], ones_f[:])

            w_r = {}
            for name, src in (("wq", wq), ("wk", wk), ("wv", wv)):
                t = pp.tile([128, CK, C], F32R, tag=f"{name}_r")
                nc.sync.dma_start(
                    t[:], src.rearrange("(k p) m -> p k m", p=128).bitcast(F32R))
                w_r[name] = t
            bq_sb = pp.tile([128, CK, 1], F32, tag="bq_sb")
            nc.sync.dma_start(bq_sb[:], bq.rearrange("(c p) o -> p c o", p=128))
            bk_sb = pp.tile([128, CK, 1], F32, tag="bk_sb")
            nc.sync.dma_start(bk_sb[:], bk.rearrange("(c p) o -> p c o", p=128))
            bv_r = pp.tile([1, C], F32R, tag="bv_r")
            nc.sync.dma_start(bv_r[:], bv.bitcast(F32R))

            # ---- stream inputs ---------------------------------------
            # xa resident (feeds QT rhs and V lhsT); pieces for queue overlap
            xa_r = pp.tile([128, CK, N], F32R, tag="xa_r")
            for ki in range(CK):
                for pc in range(4):
                    nc.sync.dma_start(
                        xa_r[:, ki, pc * NP:(pc + 1) * NP],
                        xa[ki * 128:(ki + 1) * 128,
                           pc * NP:(pc + 1) * NP].bitcast(F32R))
            # xb streamed: one [128, 1024] piece per (ki, pair)
            xb_pieces = {}
            for pc in range(4):
                for ki in range(CK):
                    t = xbp.tile([128, NP], F32R, tag="xb")
                    nc.sync.dma_start(
                        t[:], xb[ki * 128:(ki + 1) * 128,
                                 pc * NP:(pc + 1) * NP].bitcast(F32R))
                    xb_pieces[(ki, pc)] = t

            qt = pp.tile([128, CK, N], F32R, tag="qt")    # QT[c, n] co-chunked
            kt = pp.tile([128, CK, N], F32R, tag="kt")    # KT[c, m]
            v_r = pp.tile([128, CK, N], F32R, tag="v_r")  # V[m, c] -> [p, co, mb*128+j]

            # ---- phase 1: projections, V interleaved with QT/KT ------
            # jobs: 16 QT groups, then 16 KT groups; one V block between each
            qtkt_jobs = []
            for dst, w_t, b_sb, kind in ((qt, w_r["wq"], bq_sb, "qt"),
                                         (kt, w_r["wk"], bk_sb, "kt")):
                for co in range(CK):
                    for nt in range(N // NT):
                        qtkt_jobs.append((dst, w_t, b_sb, kind, co, nt))
            # QT first; KT walks n in pair order so xb pieces release in order
            qtkt_jobs = sorted(qtkt_jobs,
                               key=lambda j: (j[3] == "kt", j[5] // 2, j[4], j[5]))
            def emit_qtkt(job):
                dst, w_t, b_sb, kind, co, nt = job
                ps = sp.tile([128, NP], F32, tag="s")
                half = ps[:, 0:NT]
                for ki in range(CK):
                    if kind == "qt":
                        rhs = xa_r[:, ki, nt * NT:(nt + 1) * NT]
                    else:
                        piece = xb_pieces[(ki, nt // 2)]
                        rhs = piece[:, (nt % 2) * NT:((nt % 2) + 1) * NT]
                    nc.tensor.matmul(half, w_t[:, ki, co * 128:(co + 1) * 128],
                                     rhs, start=(ki == 0), stop=(ki == CK - 1))
                nc.vector.tensor_scalar_add(
                    dst[:, co, nt * NT:(nt + 1) * NT], half, b_sb[:, co, :])

            def emit_v(mb):
                ps = sp.tile([128, NP], F32, tag="s")
                half = ps[:, 0:C]
                for ki in range(CK):
                    nc.tensor.matmul(
                        half, xa_r[:, ki, mb * 128:(mb + 1) * 128],
                        w_r["wv"][:, ki, :], start=(ki == 0), stop=False)
                nc.tensor.matmul(half, ones_r[0:1, :], bv_r[:],
                                 start=False, stop=True)
                nc.vector.tensor_copy(
                    v_r[:, :, mb * 128:(mb + 1) * 128],
                    half.rearrange("p (c j) -> p c j", c=CK))

            # QT jobs come first (xa), KT jobs later (xb streamed in)
            for i in range(32):
                emit_qtkt(qtkt_jobs[i])
                emit_v(i)

            # ---- phase 2: attention over n-tile pairs -----------------
            for pc in range(N_PAIR):
                np_sl = slice(pc * NP, (pc + 1) * NP)
                u_ps = up.tile([128, CK, NP], F32, tag="u")
                dacc = ap_.tile([128, NP], F32R, tag="dacc")
                e_tiles = {}
                for step in range(N_MB + SKEW):
                    if step < N_MB:
                        mb = step
                        s_ps = sp.tile([128, NP], F32, tag="s")
                        for co in range(CK):
                            for ho in range(2):
                                nc.tensor.matmul(
                                    s_ps[:, ho * NT:(ho + 1) * NT],
                                    kt[:, co, mb * 128:(mb + 1) * 128],
                                    qt[:, co, (pc * 2 + ho) * NT:(pc * 2 + ho + 1) * NT],
                                    start=(co == 0), stop=(co == CK - 1))
                        e_r = ep.tile([128, NP], F32R, tag="e")
                        nc.scalar.activation(e_r[:], s_ps[:], AF.Exp)
                        e_tiles[mb] = e_r
                    if step >= SKEW:
                        mb = step - SKEW
                        e_r = e_tiles.pop(mb)
                        for co in range(CK):
                            for ho in range(2):
                                nc.tensor.matmul(
                                    u_ps[:, co, ho * NT:(ho + 1) * NT],
                                    v_r[:, co, mb * 128:(mb + 1) * 128],
                                    e_r[:, ho * NT:(ho + 1) * NT],
                                    start=(mb == 0), stop=(mb == N_MB - 1))
                        if mb == 0:
                            nc.vector.tensor_copy(dacc[:], e_r[:])
                        else:
                            nc.vector.tensor_add(dacc[:], dacc[:], e_r[:])
                # normalize: D replicated to all partitions via ones matmul
                d_ps = sp.tile([128, NP], F32, tag="s")
                for ho in range(2):
                    nc.tensor.matmul(d_ps[:, ho * NT:(ho + 1) * NT], ones_r[:],
                                     dacc[:, ho * NT:(ho + 1) * NT],
                                     start=True, stop=True)
                dinv = ap_.tile([128, NP], F32, tag="dinv")
                nc.vector.reciprocal_approx_fast(dinv[:], d_ps[:])
                for co in range(CK):
                    o_sb = op_.tile([128, NP], F32, tag="o_sb")
                    nc.vector.tensor_mul(o_sb[:], u_ps[:, co, :], dinv[:])
                    nc.sync.dma_start(out[co * 128:(co + 1) * 128, np_sl], o_sb[:])
    nc.compile()
    return nc


def _get_nc():
    global _NC_CACHE
    if _NC_CACHE is None:
        _NC_CACHE = _build()
    return _NC_CACHE


def kernel(x1, x2, Wq, bq, Wk, bk, Wv, bv):
    global LAST_RESULT
    x1 = np.asarray(x1, dtype=np.float32)
    x2 = np.asarray(x2, dtype=np.float32)
    shared = {
        "wq_t": np.ascontiguousarray(np.asarray(Wq, np.float32).T),
        "wk_t": np.ascontiguousarray(np.asarray(Wk, np.float32).T),
        "wv_t": np.ascontiguousarray(np.asarray(Wv, np.float32).T),
        "bq": np.asarray(bq, np.float32).reshape(C, 1),
        "bk": np.asarray(bk, np.float32).reshape(C, 1),
        "bv": np.asarray(bv, np.float32).reshape(1, C),
    }
    in_maps = []
    for core in range(8):
        b, s = core % B, core // B
        xs, xo = (x1, x2) if s == 0 else (x2, x1)
        in_maps.append({
            "xa": np.ascontiguousarray(xs[b].reshape(C, N)),
            "xb": np.ascontiguousarray(xo[b].reshape(C, N)),
            **shared,
        })
    nc = _get_nc()
    res = run_bass_kernel_spmd(nc, in_maps, list(range(8)))
    LAST_RESULT = res
    x1_out = np.stack([res.results[b]["o"].reshape(C, H, W) for b in range(B)])
    x2_out = np.stack([res.results[B + b]["o"].reshape(C, H, W) for b in range(B)])
    return (x1_out, x2_out)


# revision 9
# speedup vs baseline: 1.0665x; 1.0412x over previous
"""Cross-temporal attention Trainium2 (Bass/Tile) kernel.

Problem: two streams x1, x2 of shape [B=4, C=256, H=64, W=64]; tokens are the
H*W=4096 spatial positions. Per batch b and stream s:
    q_s = t_s @ Wq.T + bq ; k_s = t_s @ Wk.T + bk ; v_s = t_s @ Wv.T + bv
    out_s = softmax(q_s @ k_{3-s}.T) @ v_s            (no 1/sqrt(d) scaling)

Sharding: 8 NeuronCores, one (batch, stream) unit per core (4 batches x 2
streams). Fully SPMD — the same program runs on every core, only the input
bindings differ. No collectives.

Per-core layout trick: x[b] is already [C, N] channel-major, which is exactly
the transposed token matrix. All intermediates stay transposed:
    QT = Wq @ X + bq   [C, N]      (PE: lhsT = Wq^T chunks, rhs = X chunks)
    KT = Wk @ Xo + bk  [C, N]
    V  = X^T @ Wv^T + bv  [N, C]   (PE: lhsT = X chunks, rhs = Wv^T)
    ST = KT^T-block @ QT = scores^T  [m, n] blocks   (softmax over m = partitions)
    E  = exp(ST)   (no max subtraction: |logits| < ~40 << 88, fp32-safe)
    U  = accum_m V^T-block @ E  -> [C, n] unnormalized out^T
    D  = column sums of E (ones-matmul replicates to all partitions)
    OT = U / D     [C, N] == x_out[b] flattened. No transposes anywhere.

All matmuls run in float32r (TF32) at 1 cycle/row; data is DMA'd straight into
float32r tiles (PE rounds internally) or produced by compute ops with float32r
output dtype.

v2 notes (driven by NTFF trace of v1, 355us):
 - attention processes n-tile PAIRS (1024 columns): one exp + one dacc add per
   key block covers both tiles (fewer, wider ACT/DVE ops).
 - V-projection matmuls (256-free, LDWEIGHTS-bound) are interleaved between
   QT/KT groups so the PE never micro-idles long enough to re-arm the HAM
   throttle (v1 lost ~37us to a K=4/8 window at the phase boundary).
 - input DMAs split into 512KB pieces for queue parallelism (v1 stalled ~20us
   on the first 2MB DMA); xb is streamed through a small pool, not resident.
 - reciprocal -> reciprocal_approx_fast (denominators need ~18 bits).
"""

import numpy as np

import concourse.bacc as bacc
import concourse.mybir as mybir
import concourse.tile as tile
from concourse.bass_utils import run_bass_kernel_spmd

F32 = mybir.dt.float32
F32R = mybir.dt.float32r
AF = mybir.ActivationFunctionType

B, C, H, W = 4, 256, 64, 64
N = H * W            # 4096 tokens
CK = C // 128        # 2 channel chunks of 128
NT = 512             # attention n-tile (query block, free dim)
NP = 1024            # n-tile pair width
N_PAIR = N // NP     # 4
MB = 128             # key/value block (partition block)
N_MB = N // MB       # 32
SKEW = 2             # software-pipeline skew between S and U matmuls

_NC_CACHE = None
LAST_RESULT = None   # BassKernelResults of the most recent kernel() call


def _build():
    nc = bacc.Bacc("TRN2", target_bir_lowering=False, debug=False)

    xa = nc.dram_tensor("xa", [C, N], F32, kind="ExternalInput").ap()
    xb = nc.dram_tensor("xb", [C, N], F32, kind="ExternalInput").ap()
    wq = nc.dram_tensor("wq_t", [C, C], F32, kind="ExternalInput").ap()
    wk = nc.dram_tensor("wk_t", [C, C], F32, kind="ExternalInput").ap()
    wv = nc.dram_tensor("wv_t", [C, C], F32, kind="ExternalInput").ap()
    bq = nc.dram_tensor("bq", [C, 1], F32, kind="ExternalInput").ap()
    bk = nc.dram_tensor("bk", [C, 1], F32, kind="ExternalInput").ap()
    bv = nc.dram_tensor("bv", [1, C], F32, kind="ExternalInput").ap()
    out = nc.dram_tensor("o", [C, N], F32, kind="ExternalOutput").ap()

    with tile.TileContext(nc) as tc:
        with tc.tile_pool(name="persist", bufs=1) as pp, \
             tc.tile_pool(name="xbs", bufs=4) as xbp, \
             tc.tile_pool(name="os", bufs=2) as op_, \
             tc.tile_pool(name="s_ps", bufs=2, space="PSUM") as sp, \
             tc.tile_pool(name="u_ps", bufs=1, space="PSUM") as up, \
             tc.tile_pool(name="e_sb", bufs=3) as ep, \
             tc.tile_pool(name="acc", bufs=1) as ap_:
            # ---- constants & parameters -------------------------------
            ones_f = pp.tile([128, 128], F32, tag="ones_f")
            nc.vector.memset(ones_f[:], 1.0)
            ones_r = pp.tile([128, 128], F32R, tag="ones_r")
            nc.vector.tensor_copy(ones_r[:], ones_f[:])

            # HAM warmup: ~14 dependency-free matmuls run back-to-back during
            # the input DMA wait, arming the PE clock gate (K=8/8) so phase 1
            # doesn't execute at the cold 1.2 GHz half-clock (v2 lost ~38us).
            warm_src = pp.tile([128, NT], F32R, tag="warm_src")
            nc.vector.tensor_copy(warm_src[:, 0:128], ones_f[:])
            for j in range(1, 4):
                nc.vector.tensor_copy(
                    warm_src[:, j * 128:(j + 1) * 128], ones_f[:])
            warm_ps = sp.tile([128, NP], F32, tag="s")
            for it in range(14):
                nc.tensor.matmul(warm_ps[:, 0:NT], ones_r[:], warm_src[:],
                                 start=(it == 0), stop=(it == 13))

            w_r = {}
            for name, src in (("wq", wq), ("wk", wk), ("wv", wv)):
                t = pp.tile([128, CK, C], F32R, tag=f"{name}_r")
                nc.sync.dma_start(
                    t[:], src.rearrange("(k p) m -> p k m", p=128).bitcast(F32R))
                w_r[name] = t
            bq_sb = pp.tile([128, CK, 1], F32, tag="bq_sb")
            nc.sync.dma_start(bq_sb[:], bq.rearrange("(c p) o -> p c o", p=128))
            bk_sb = pp.tile([128, CK, 1], F32, tag="bk_sb")
            nc.sync.dma_start(bk_sb[:], bk.rearrange("(c p) o -> p c o", p=128))
            bv_r = pp.tile([1, C], F32R, tag="bv_r")
            nc.sync.dma_start(bv_r[:], bv.bitcast(F32R))

            # ---- stream inputs ---------------------------------------
            # xa resident as separate piece tiles (per-piece deps let QT/V
            # start as soon as the first piece lands, not after all 8 DMAs)
            xa_pieces = {}
            for pc in range(4):
                for ki in range(CK):
                    t = pp.tile([128, NP], F32R, tag=f"xa_{ki}_{pc}")
                    nc.sync.dma_start(
                        t[:], xa[ki * 128:(ki + 1) * 128,
                                 pc * NP:(pc + 1) * NP].bitcast(F32R))
                    xa_pieces[(ki, pc)] = t
            # xb streamed: one [128, 1024] piece per (ki, pair)
            xb_pieces = {}
            for pc in range(4):
                for ki in range(CK):
                    t = xbp.tile([128, NP], F32R, tag="xb")
                    nc.sync.dma_start(
                        t[:], xb[ki * 128:(ki + 1) * 128,
                                 pc * NP:(pc + 1) * NP].bitcast(F32R))
                    xb_pieces[(ki, pc)] = t

            qt = pp.tile([128, CK, N], F32R, tag="qt")    # QT[c, n] co-chunked
            kt = pp.tile([128, CK, N], F32R, tag="kt")    # KT[c, m]
            v_r = pp.tile([128, CK, N], F32R, tag="v_r")  # V[m, c] -> [p, co, mb*128+j]

            # ---- phase 1: projections, V interleaved with QT/KT ------
            # jobs: 16 QT groups, then 16 KT groups; one V block between each
            qtkt_jobs = []
            for dst, w_t, b_sb, kind in ((qt, w_r["wq"], bq_sb, "qt"),
                                         (kt, w_r["wk"], bk_sb, "kt")):
                for co in range(CK):
                    for nt in range(N // NT):
                        qtkt_jobs.append((dst, w_t, b_sb, kind, co, nt))
            # QT first; KT walks n in pair order so xb pieces release in order
            qtkt_jobs = sorted(qtkt_jobs,
                               key=lambda j: (j[3] == "kt", j[5] // 2, j[4], j[5]))
            def emit_qtkt(job):
                dst, w_t, b_sb, kind, co, nt = job
                ps = sp.tile([128, NP], F32, tag="s")
                half = ps[:, 0:NT]
                for ki in range(CK):
                    if kind == "qt":
                        piece = xa_pieces[(ki, nt // 2)]
                    else:
                        piece = xb_pieces[(ki, nt // 2)]
                    rhs = piece[:, (nt % 2) * NT:((nt % 2) + 1) * NT]
                    nc.tensor.matmul(half, w_t[:, ki, co * 128:(co + 1) * 128],
                                     rhs, start=(ki == 0), stop=(ki == CK - 1))
                nc.vector.tensor_scalar_add(
                    dst[:, co, nt * NT:(nt + 1) * NT], half, b_sb[:, co, :])

            def emit_v(mb):
                ps = sp.tile([128, NP], F32, tag="s")
                half = ps[:, 0:C]
                for ki in range(CK):
                    piece = xa_pieces[(ki, mb // 8)]
                    nc.tensor.matmul(
                        half, piece[:, (mb % 8) * 128:((mb % 8) + 1) * 128],
                        w_r["wv"][:, ki, :], start=(ki == 0), stop=False)
                nc.tensor.matmul(half, ones_r[0:1, :], bv_r[:],
                                 start=False, stop=True)
                nc.vector.tensor_copy(
                    v_r[:, :, mb * 128:(mb + 1) * 128],
                    half.rearrange("p (c j) -> p c j", c=CK))

            # QT jobs come first (xa), KT jobs later (xb streamed in)
            for i in range(32):
                emit_qtkt(qtkt_jobs[i])
                emit_v(i)

            # ---- phase 2: attention over n-tile pairs -----------------
            for pc in range(N_PAIR):
                np_sl = slice(pc * NP, (pc + 1) * NP)
                u_ps = up.tile([128, CK, NP], F32, tag="u")
                dacc = ap_.tile([128, NP], F32R, tag="dacc")
                e_tiles = {}
                for step in range(N_MB + SKEW):
                    if step < N_MB:
                        mb = step
                        s_ps = sp.tile([128, NP], F32, tag="s")
                        for co in range(CK):
                            for ho in range(2):
                                nc.tensor.matmul(
                                    s_ps[:, ho * NT:(ho + 1) * NT],
                                    kt[:, co, mb * 128:(mb + 1) * 128],
                                    qt[:, co, (pc * 2 + ho) * NT:(pc * 2 + ho + 1) * NT],
                                    start=(co == 0), stop=(co == CK - 1))
                        e_r = ep.tile([128, NP], F32R, tag="e")
                        nc.scalar.activation(e_r[:], s_ps[:], AF.Exp)
                        e_tiles[mb] = e_r
                    if step >= SKEW:
                        mb = step - SKEW
                        e_r = e_tiles.pop(mb)
                        for co in range(CK):
                            for ho in range(2):
                                nc.tensor.matmul(
                                    u_ps[:, co, ho * NT:(ho + 1) * NT],
                                    v_r[:, co, mb * 128:(mb + 1) * 128],
                                    e_r[:, ho * NT:(ho + 1) * NT],
                                    start=(mb == 0), stop=(mb == N_MB - 1))
                        if mb == 0:
                            nc.vector.tensor_copy(dacc[:], e_r[:])
                        else:
                            nc.vector.tensor_add(dacc[:], dacc[:], e_r[:])
                # normalize: D replicated to all partitions via ones matmul
                d_ps = sp.tile([128, NP], F32, tag="s")
                for ho in range(2):
                    nc.tensor.matmul(d_ps[:, ho * NT:(ho + 1) * NT], ones_r[:],
                                     dacc[:, ho * NT:(ho + 1) * NT],
                                     start=True, stop=True)
                dinv = ap_.tile([128, NP], F32, tag="dinv")
                nc.vector.reciprocal_approx_fast(dinv[:], d_ps[:])
                for co in range(CK):
                    o_sb = op_.tile([128, NP], F32, tag="o_sb")
                    nc.vector.tensor_mul(o_sb[:], u_ps[:, co, :], dinv[:])
                    nc.sync.dma_start(out[co * 128:(co + 1) * 128, np_sl], o_sb[:])
    nc.compile()
    return nc


def _get_nc():
    global _NC_CACHE
    if _NC_CACHE is None:
        _NC_CACHE = _build()
    return _NC_CACHE


def kernel(x1, x2, Wq, bq, Wk, bk, Wv, bv):
    global LAST_RESULT
    x1 = np.asarray(x1, dtype=np.float32)
    x2 = np.asarray(x2, dtype=np.float32)
    shared = {
        "wq_t": np.ascontiguousarray(np.asarray(Wq, np.float32).T),
        "wk_t": np.ascontiguousarray(np.asarray(Wk, np.float32).T),
        "wv_t": np.ascontiguousarray(np.asarray(Wv, np.float32).T),
        "bq": np.asarray(bq, np.float32).reshape(C, 1),
        "bk": np.asarray(bk, np.float32).reshape(C, 1),
        "bv": np.asarray(bv, np.float32).reshape(1, C),
    }
    in_maps = []
    for core in range(8):
        b, s = core % B, core // B
        xs, xo = (x1, x2) if s == 0 else (x2, x1)
        in_maps.append({
            "xa": np.ascontiguousarray(xs[b].reshape(C, N)),
            "xb": np.ascontiguousarray(xo[b].reshape(C, N)),
            **shared,
        })
    nc = _get_nc()
    res = run_bass_kernel_spmd(nc, in_maps, list(range(8)))
    LAST_RESULT = res
    x1_out = np.stack([res.results[b]["o"].reshape(C, H, W) for b in range(B)])
    x2_out = np.stack([res.results[B + b]["o"].reshape(C, H, W) for b in range(B)])
    return (x1_out, x2_out)
